# revision 8
# baseline (speedup 1.0000x reference)
"""CategoryDense (nn_CategoryDense) TRN2 Bass kernel — bf16, host-side
transpose layout, single-ring DMA.

out[b, c, o] = sum_i x[b, c, i] * kernel[0, c, i, o] + bias[0, c, o]
x: [8192, 64, 64] f32; kernel: [1, 64, 64, 64]; bias: [1, 64, 64].

Data-parallel over 8 NeuronCores: batch dim sharded 1024 rows/core,
weights + bias replicated; no cross-core communication.

The problem is HBM-bound: ~17.8 MB of per-core traffic. All device I/O
is bf16 (host casts x down and the result back up; rel-err budget 2e-2,
~4.3e-3 measured), halving DMA bytes vs f32.

The host upload stores x pre-transposed per 128-row b-tile as
xt[t, p, j, b] = x[128t+b, 128j+p] (p = contraction index of category
pair j), so matmul lhsT tiles stream straight from HBM — no PE
transposes.

Device structure per core (8 b-tiles x 4 [128,1024] PSUM groups):
  - PE runs ONLY the 256 [128,128] bf16 matmuls vs the block-diagonal
    weight stacks.  (The DMA feed paces PE, which keeps it at its mid
    p-state ~0.83 ns/cycle, so every extra PE cycle costs double —
    bias never touches PE.)
  - Bias: ~3/8 of groups are ACT-drained: ACT pre-fills the PSUM group
    with broadcast bias two groups ahead (matmuls accumulate with
    start=False) and drains with a plain cast-copy.  The other groups
    drain via DVE tensor_add with the bias inline.  This splits the
    mandatory PSUM->SBUF traffic across both engines (~29 us each).
  - ALL x loads and out stores ride the single SP HWDGE ring,
    interleaved so stores trail their drains by >1 tile and never
    head-of-line block loads (and never compete for SDMA packet slots
    at mismatched packet sizes, which throttled split-ring variants).
"""

from contextlib import ExitStack

import numpy as np
import ml_dtypes

import concourse.bass as bass  # noqa: F401  (engine namespaces live on nc)
import concourse.mybir as mybir
import concourse.tile as tile
from concourse import bacc
from concourse.bass_utils import run_bass_kernel_spmd

F32 = mybir.dt.float32
BF16 = mybir.dt.bfloat16
NP_BF16 = ml_dtypes.bfloat16

N_CORES = 8
B, C, IN, OUT = 8192, 64, 64, 64
B_SHARD = B // N_CORES
N_PAIRS = C // 2          # 32 category pairs; one [128,128] matmul each
CI = C * IN               # 4096
CO = C * OUT              # 4096
N_BTILES = B_SHARD // 128  # 8
GROUPS = 4                # [128,1024] PSUM groups per b-tile (8 pairs each)
SKEW = 2                  # matmul emission lag behind psum alloc/prefill
# Units g with g % 8 in ACT_PHASE are ACT-drained (bias prefilled by
# ACT), the rest DVE-drained (bias added inline).  12/32 on ACT.
ACT_PHASE = (1, 4, 6)


def _build_nc(b_shard=B_SHARD):
    n_btiles = b_shard // 128
    total = n_btiles * GROUPS
    nc = bacc.Bacc("TRN2", target_bir_lowering=False, debug=False)
    # Host-pre-transposed x: xt[t, p, j, b] = x[128t+b, 128j+p].
    xt = nc.dram_tensor("xt", [n_btiles, 128, N_PAIRS, 128], BF16,
                        kind="ExternalInput").ap()
    # Host-prepared block-diagonal weight stacks (see kernel() below).
    wall = nc.dram_tensor("wall", [128, N_PAIRS, 128], BF16,
                          kind="ExternalInput").ap()
    bias1 = nc.dram_tensor("bias1", [1, CO], BF16, kind="ExternalInput").ap()
    out = nc.dram_tensor("out", [b_shard, C, OUT], BF16,
                         kind="ExternalOutput").ap()

    out_t = out.rearrange("(t p) c o -> t p (c o)", p=128)

    with tile.TileContext(nc) as tc, ExitStack() as ctx:
        const_pool = ctx.enter_context(tc.tile_pool(name="const", bufs=1))
        x_pool = ctx.enter_context(tc.tile_pool(name="x", bufs=4))
        out_pool = ctx.enter_context(tc.tile_pool(name="out", bufs=3))
        # Four 2-bank PSUM tiles = all 8 banks.
        psum_o = ctx.enter_context(
            tc.tile_pool(name="psum_o", bufs=4, space="PSUM"))

        # Constants on the ACT HWDGE ring: tiny bias first, then weights
        # chunked so matmul group g only waits on its own chunks.
        bias_row = const_pool.tile([1, CO], BF16)
        nc.scalar.dma_start(bias_row[:], bias1[:])
        w_all = const_pool.tile([128, N_PAIRS, 128], BF16)
        for k in range(8):
            nc.scalar.dma_start(w_all[:, 4 * k:4 * (k + 1)],
                                wall[:, 4 * k:4 * (k + 1)])
        # Bias replicated to all 128 partitions on-chip (SWDGE).
        bias_sb = const_pool.tile([128, CO], BF16)
        nc.gpsimd.partition_broadcast(bias_sb[:], bias_row[:], channels=128)

        def load_tile(t):
            x_sb = x_pool.tile([128, N_PAIRS, 128], BF16, tag="x_sb",
                               name=f"x_sb_{t}")
            for h in range(2):
                nc.sync.dma_start(x_sb[:, h * 16:(h + 1) * 16],
                                  xt[t][:, h * 16:(h + 1) * 16])
            return x_sb

        xs = {t: load_tile(t) for t in range(3)}
        o_tiles = {}
        ps_tiles = {}

        for step in range(total + SKEW):
            if step < total:
                g = step
                t, u = divmod(g, GROUPS)
                if u == 0 and t + 3 < n_btiles:
                    xs[t + 3] = load_tile(t + 3)
                ps_o = psum_o.tile([128, 1024], F32, tag="ps_o",
                                   name=f"ps_o_{g}")
                ps_tiles[g] = ps_o
                if g % 8 in ACT_PHASE:
                    # ACT pre-fills the group with bias; matmuls then
                    # accumulate on top (start=False).
                    c0 = u * 1024
                    nc.scalar.copy(ps_o[:], bias_sb[:, c0:c0 + 1024])
            if step >= SKEW:
                g = step - SKEW
                t, u = divmod(g, GROUPS)
                act_group = (g % 8 in ACT_PHASE)
                c0 = u * 1024
                if u == 0:
                    o_tiles[t] = out_pool.tile([128, CO], BF16, tag="o_sb",
                                               name=f"o_sb_{t}")
                o_sb = o_tiles[t]
                ps_o = ps_tiles.pop(g)
                for j in range(8):
                    p = 8 * u + j  # pair index within tile
                    nc.tensor.matmul(ps_o[:, j * 128:(j + 1) * 128],
                                     lhsT=xs[t][:, p],
                                     rhs=w_all[:, p],
                                     start=not act_group,
                                     stop=(not act_group) or (j % 4 == 3),
                                     skip_group_check=True)
                if act_group:
                    nc.scalar.copy(o_sb[:, c0:c0 + 1024], ps_o[:])
                else:
                    nc.vector.tensor_add(out=o_sb[:, c0:c0 + 1024],
                                         in0=ps_o[:],
                                         in1=bias_sb[:, c0:c0 + 1024])
                # Stores ride the same SP ring, after the NEXT tiles'
                # loads in FIFO order, so they never block a load that
                # isn't already 3 tiles ahead.
                if t < n_btiles - 1:
                    if u == GROUPS - 1:
                        for h in range(2):
                            nc.sync.dma_start(
                                out_t[t][:, h * 2048:(h + 1) * 2048],
                                o_sb[:, h * 2048:(h + 1) * 2048])
                else:
                    nc.sync.dma_start(out_t[t][:, c0:c0 + 1024],
                                      o_sb[:, c0:c0 + 1024])

    nc.compile()
    return nc


_NC_CACHE = {}


def _get_nc():
    if "nc" not in _NC_CACHE:
        _NC_CACHE["nc"] = _build_nc()
    return _NC_CACHE["nc"]


def _install_ntff_shim():
    """Profiling only: register the axon NTFF hook under antenv.axon_hooks.

    The container's antenv stub lacks axon_hooks, so bass_utils'
    `from antenv.axon_hooks import get_axon_ntff_profile_hook` raises on
    trace=True runs. Recreate the module from trn_agent_boot's ctypes hook.
    """
    import sys
    import types

    if "antenv.axon_hooks" in sys.modules:
        return
    from trn_agent_boot.trn_boot import _ntff_profile_via_ctypes

    hook = _ntff_profile_via_ctypes("/opt/axon/libaxon_pjrt.so")
    mod = types.ModuleType("antenv.axon_hooks")
    mod.get_axon_ntff_profile_hook = lambda: hook
    mod.set_axon_ntff_profile_hook = lambda h: None
    sys.modules["antenv.axon_hooks"] = mod
    import antenv

    antenv.axon_hooks = mod


def kernel(x, kernel, bias, _trace=False, _trace_kwargs=None):
    x = np.ascontiguousarray(x, dtype=np.float32)
    kernel = np.ascontiguousarray(kernel, dtype=np.float32)
    bias = np.ascontiguousarray(bias, dtype=np.float32)
    assert x.shape == (B, C, IN)

    if _trace:
        _install_ntff_shim()
    nc = _get_nc()

    # bf16 cast + per-b-tile transpose: xt[s, t, p, j, b] = shard s's
    # x[128t+b, 128j+p], so lhsT tiles stream straight from HBM.
    xb = x.reshape(N_CORES, N_BTILES, 128, N_PAIRS, 128).astype(NP_BF16)
    xtb = np.ascontiguousarray(xb.transpose(0, 1, 4, 3, 2))
    # Block-diagonal bf16 weight stacks: wall[p, j, :] holds cat 2j's
    # [i, o] block at [0:64, 0:64] and cat 2j+1's at [64:128, 64:128].
    wall = np.zeros((128, N_PAIRS, 128), dtype=np.float32)
    wall[0:IN, :, 0:OUT] = kernel[0, 0::2].transpose(1, 0, 2)
    wall[IN:128, :, OUT:128] = kernel[0, 1::2].transpose(1, 0, 2)
    wall = wall.astype(NP_BF16)
    bias1 = bias.reshape(1, CO).astype(NP_BF16)
    in_maps = [
        {
            "xt": xtb[i],
            "wall": wall,
            "bias1": bias1,
        }
        for i in range(N_CORES)
    ]
    res = run_bass_kernel_spmd(
        nc, in_maps, core_ids=list(range(N_CORES)),
        trace=_trace, **(_trace_kwargs or {})
    )
    out = np.concatenate(
        [np.asarray(res.results[i]["out"]) for i in range(N_CORES)], axis=0
    ).astype(np.float32)
    if _trace:
        _NC_CACHE["last_results"] = res
    return out


# revision 11
# speedup vs baseline: 1.0622x; 1.0622x over previous
"""CategoryDense (nn_CategoryDense) TRN2 Bass kernel — bf16, host-side
transpose layout, single-ring DMA.

out[b, c, o] = sum_i x[b, c, i] * kernel[0, c, i, o] + bias[0, c, o]
x: [8192, 64, 64] f32; kernel: [1, 64, 64, 64]; bias: [1, 64, 64].

Data-parallel over 8 NeuronCores: batch dim sharded 1024 rows/core,
weights + bias replicated; no cross-core communication.

The problem is HBM-bound: ~17.8 MB of per-core traffic. All device I/O
is bf16 (host casts x down and the result back up; rel-err budget 2e-2,
~4.3e-3 measured), halving DMA bytes vs f32.

The host upload stores x pre-transposed per 128-row b-tile as
xt[t, p, j, b] = x[128t+b, 128j+p] (p = contraction index of category
pair j), so matmul lhsT tiles stream straight from HBM — no PE
transposes.

Device structure per core (8 b-tiles x 4 [128,1024] PSUM groups):
  - PE runs ONLY the 256 [128,128] bf16 matmuls vs the block-diagonal
    weight stacks.  (The DMA feed paces PE, which keeps it at its mid
    p-state ~0.83 ns/cycle, so every extra PE cycle costs double —
    bias never touches PE.)
  - Bias: ~3/8 of groups are ACT-drained: ACT pre-fills the PSUM group
    with broadcast bias two groups ahead (matmuls accumulate with
    start=False) and drains with a plain cast-copy.  The other groups
    drain via DVE tensor_add with the bias inline.  This splits the
    mandatory PSUM->SBUF traffic across both engines (~29 us each).
  - ALL x loads and out stores ride the single SP HWDGE ring,
    interleaved so stores trail their drains by >1 tile and never
    head-of-line block loads (and never compete for SDMA packet slots
    at mismatched packet sizes, which throttled split-ring variants).
"""

from contextlib import ExitStack

import numpy as np
import ml_dtypes

import concourse.bass as bass  # noqa: F401  (engine namespaces live on nc)
import concourse.mybir as mybir
import concourse.tile as tile
from concourse import bacc
from concourse.bass_utils import run_bass_kernel_spmd

F32 = mybir.dt.float32
BF16 = mybir.dt.bfloat16
NP_BF16 = ml_dtypes.bfloat16

N_CORES = 8
B, C, IN, OUT = 8192, 64, 64, 64
B_SHARD = B // N_CORES
N_PAIRS = C // 2          # 32 category pairs; one [128,128] matmul each
CI = C * IN               # 4096
CO = C * OUT              # 4096
N_BTILES = B_SHARD // 128  # 8
GROUPS = 4                # [128,1024] PSUM groups per b-tile (8 pairs each)
SKEW = 2                  # matmul emission lag behind psum alloc/prefill
# Units g with g % 8 in ACT_PHASE are ACT-drained (bias prefilled by
# ACT), the rest DVE-drained (bias added inline).  12/32 on ACT.
ACT_PHASE = (1, 4, 6)


def _build_nc(b_shard=B_SHARD):
    n_btiles = b_shard // 128
    total = n_btiles * GROUPS
    nc = bacc.Bacc("TRN2", target_bir_lowering=False, debug=False)
    # Host-pre-transposed x: xt[t, p, j, b] = x[128t+b, 128j+p].
    xt = nc.dram_tensor("xt", [n_btiles, 128, N_PAIRS, 128], BF16,
                        kind="ExternalInput").ap()
    # Host-prepared block-diagonal weight stacks (see kernel() below).
    wall = nc.dram_tensor("wall", [128, N_PAIRS, 128], BF16,
                          kind="ExternalInput").ap()
    bias1 = nc.dram_tensor("bias1", [1, CO], BF16, kind="ExternalInput").ap()
    out = nc.dram_tensor("out", [b_shard, C, OUT], BF16,
                         kind="ExternalOutput").ap()

    out_t = out.rearrange("(t p) c o -> t p (c o)", p=128)

    with tile.TileContext(nc) as tc, ExitStack() as ctx:
        const_pool = ctx.enter_context(tc.tile_pool(name="const", bufs=1))
        x_pool = ctx.enter_context(tc.tile_pool(name="x", bufs=4))
        out_pool = ctx.enter_context(tc.tile_pool(name="out", bufs=3))
        # Four 2-bank PSUM tiles = all 8 banks.
        psum_o = ctx.enter_context(
            tc.tile_pool(name="psum_o", bufs=4, space="PSUM"))

        # Constants on the ACT HWDGE ring.  Few, large DMAs: each
        # dma_start costs ~650ns of sequencer + sem-lane time, and the 8
        # HWDGE completion lanes are shared with the load ring, so a
        # chain of small const DMAs head-of-line blocks the x loads.
        bias_row = const_pool.tile([1, CO], BF16)
        nc.scalar.dma_start(bias_row[:], bias1[:])
        w_all = const_pool.tile([128, N_PAIRS, 128], BF16)
        for k in range(2):
            nc.scalar.dma_start(w_all[:, 16 * k:16 * (k + 1)],
                                wall[:, 16 * k:16 * (k + 1)])
        # Bias replicated to all 128 partitions on-chip (SWDGE).
        bias_sb = const_pool.tile([128, CO], BF16)
        nc.gpsimd.partition_broadcast(bias_sb[:], bias_row[:], channels=128)
        # Rank-1 ones column for the PE bias pre-load on ACT groups.
        ones = const_pool.tile([1, 128], BF16)
        nc.gpsimd.memset(ones[:], 1.0)

        def load_tile(t):
            x_sb = x_pool.tile([128, N_PAIRS, 128], BF16, tag="x_sb",
                               name=f"x_sb_{t}")
            if t == 0:
                for h in range(2):
                    nc.sync.dma_start(x_sb[:, h * 16:(h + 1) * 16],
                                      xt[t][:, h * 16:(h + 1) * 16])
            else:
                nc.sync.dma_start(x_sb[:], xt[t])
            return x_sb

        xs = {t: load_tile(t) for t in range(3)}
        o_tiles = {}
        ps_tiles = {}

        for step in range(total + SKEW):
            if step < total:
                g = step
                t, u = divmod(g, GROUPS)
                if u == 0 and t + 3 < n_btiles:
                    xs[t + 3] = load_tile(t + 3)
                ps_o = psum_o.tile([128, 1024], F32, tag="ps_o",
                                   name=f"ps_o_{g}")
                ps_tiles[g] = ps_o
            if step >= SKEW:
                g = step - SKEW
                t, u = divmod(g, GROUPS)
                act_group = (g % 8 in ACT_PHASE)
                c0 = u * 1024
                if u == 0:
                    o_tiles[t] = out_pool.tile([128, CO], BF16, tag="o_sb",
                                               name=f"o_sb_{t}")
                o_sb = o_tiles[t]
                ps_o = ps_tiles.pop(g)
                if act_group:
                    # PE pre-loads bias per bank: ones[128] x bias[512].
                    for h in range(2):
                        nc.tensor.matmul(
                            ps_o[:, h * 512:(h + 1) * 512], lhsT=ones[:],
                            rhs=bias_row[0:1, c0 + h * 512:c0 + (h + 1) * 512],
                            start=True, stop=False, skip_group_check=True)
                for j in range(8):
                    p = 8 * u + j  # pair index within tile
                    nc.tensor.matmul(ps_o[:, j * 128:(j + 1) * 128],
                                     lhsT=xs[t][:, p],
                                     rhs=w_all[:, p],
                                     start=not act_group,
                                     stop=(not act_group) or (j % 4 == 3),
                                     skip_group_check=True)
                if act_group:
                    nc.scalar.copy(o_sb[:, c0:c0 + 1024], ps_o[:])
                else:
                    nc.vector.tensor_add(out=o_sb[:, c0:c0 + 1024],
                                         in0=ps_o[:],
                                         in1=bias_sb[:, c0:c0 + 1024])
                # Stores on the SWDGE (gpsimd) ring: whole tiles (their
                # 4KB-per-partition descriptors also lose the SDMA
                # packet round-robin against the 8KB load descriptors,
                # keeping the pipeline-pacing loads fast); quarters for
                # the final tile so the tail drains early.
                if t < n_btiles - 1:
                    if u == GROUPS - 1:
                        nc.gpsimd.dma_start(out_t[t], o_sb[:])
                else:
                    nc.gpsimd.dma_start(out_t[t][:, c0:c0 + 1024],
                                        o_sb[:, c0:c0 + 1024])

    nc.compile()
    return nc


_NC_CACHE = {}


def _get_nc():
    if "nc" not in _NC_CACHE:
        _NC_CACHE["nc"] = _build_nc()
    return _NC_CACHE["nc"]


def _install_ntff_shim():
    """Profiling only: register the axon NTFF hook under antenv.axon_hooks.

    The container's antenv stub lacks axon_hooks, so bass_utils'
    `from antenv.axon_hooks import get_axon_ntff_profile_hook` raises on
    trace=True runs. Recreate the module from trn_agent_boot's ctypes hook.
    """
    import sys
    import types

    if "antenv.axon_hooks" in sys.modules:
        return
    from trn_agent_boot.trn_boot import _ntff_profile_via_ctypes

    hook = _ntff_profile_via_ctypes("/opt/axon/libaxon_pjrt.so")
    mod = types.ModuleType("antenv.axon_hooks")
    mod.get_axon_ntff_profile_hook = lambda: hook
    mod.set_axon_ntff_profile_hook = lambda h: None
    sys.modules["antenv.axon_hooks"] = mod
    import antenv

    antenv.axon_hooks = mod


def kernel(x, kernel, bias, _trace=False, _trace_kwargs=None):
    x = np.ascontiguousarray(x, dtype=np.float32)
    kernel = np.ascontiguousarray(kernel, dtype=np.float32)
    bias = np.ascontiguousarray(bias, dtype=np.float32)
    assert x.shape == (B, C, IN)

    if _trace:
        _install_ntff_shim()
    nc = _get_nc()

    # bf16 cast + per-b-tile transpose: xt[s, t, p, j, b] = shard s's
    # x[128t+b, 128j+p], so lhsT tiles stream straight from HBM.
    xb = x.reshape(N_CORES, N_BTILES, 128, N_PAIRS, 128).astype(NP_BF16)
    xtb = np.ascontiguousarray(xb.transpose(0, 1, 4, 3, 2))
    # Block-diagonal bf16 weight stacks: wall[p, j, :] holds cat 2j's
    # [i, o] block at [0:64, 0:64] and cat 2j+1's at [64:128, 64:128].
    wall = np.zeros((128, N_PAIRS, 128), dtype=np.float32)
    wall[0:IN, :, 0:OUT] = kernel[0, 0::2].transpose(1, 0, 2)
    wall[IN:128, :, OUT:128] = kernel[0, 1::2].transpose(1, 0, 2)
    wall = wall.astype(NP_BF16)
    bias1 = bias.reshape(1, CO).astype(NP_BF16)
    in_maps = [
        {
            "xt": xtb[i],
            "wall": wall,
            "bias1": bias1,
        }
        for i in range(N_CORES)
    ]
    res = run_bass_kernel_spmd(
        nc, in_maps, core_ids=list(range(N_CORES)),
        trace=_trace, **(_trace_kwargs or {})
    )
    out = np.concatenate(
        [np.asarray(res.results[i]["out"]) for i in range(N_CORES)], axis=0
    ).astype(np.float32)
    if _trace:
        _NC_CACHE["last_results"] = res
    return out
